# revision 3
# baseline (speedup 1.0000x reference)
"""LSTM discriminator kernel for Trainium2 (8 NeuronCores, SPMD data-parallel).

Problem: B=4096, T=256, D=128, H=32 LSTM + final linear to 2 classes.
Sharding: batch split across 8 cores (512 rows each); weights replicated.

Per-core design v3 (batch-major, 2 pipelined batch streams of 256):
  - Per stream, gates PSUM [128b, 2ch x 128g] built by 5 matmuls: one K=1
    ones-row matmul adds the bias, then per chunk a feed matmul (feed
    chunk stationary) and a K=32 recurrence matmul off the stacked
    hT4 [2ch*32h, 128b] state.
  - One sigmoid ACT covers all gates (tanh folded via doubled g-rows);
    h kept as h/2 (W_hh, W_out doubled) so (sig-0.5)*o gives it directly.
  - c update: u=(g'-0.5)*i [DVE], v=f*c [GPSIMD or DVE], c=2u+v [DVE];
    h/2=(sig(2c)-0.5)*o [DVE].
  - ONE PE transpose per stream ([128b, 64] -> [64, 128b]) yields the
    stacked hT4 for the next step; single PSUM->SBUF copy [64, 128].
  - Two independent batch streams phase-offset so ACT/DVE/PE/GPSIMD
    overlap across streams.
"""

import numpy as np
import ml_dtypes

import concourse.bass as bass
import concourse.mybir as mybir
from concourse.tile import TileContext
from concourse.bass_utils import run_bass_kernel_spmd

F32 = mybir.dt.float32
BF16 = mybir.dt.bfloat16
FP16 = mybir.dt.float16
BF = ml_dtypes.bfloat16

B, T, D, H = 4096, 256, 128, 32
G = 4 * H          # 128 gate rows
NCORES = 8
BC = B // NCORES   # 512 batch per core
NS = 2             # pipelined batch streams
NCH = BC // NS // 128   # chunks of 128 per stream (=2)
FD = BC // NS      # 256 batch per stream
TCHUNK = 8         # timesteps per feed DMA (1 MiB transfers)
V_ON_GPSIMD = False

SIG = mybir.ActivationFunctionType.Sigmoid
MULT = mybir.AluOpType.mult
SUB = mybir.AluOpType.subtract
ADD = mybir.AluOpType.add

LAST_RESULTS = None


# ---------------------------------------------------------------- legalize ---
_lgl_ctr = [0]


def _legalize_sync_waits(nc):
    for fn in nc.m.functions:
        for blk in fn.blocks:
            new = []
            changed = False
            for inst in blk.instructions:
                si = getattr(inst, "sync_info", None)
                waits = list(si.on_wait) if (si is not None and si.on_wait) else []
                if len(waits) > 1:
                    for w in waits[:-1]:
                        _lgl_ctr[0] += 1
                        new.append(mybir.InstNoOp(
                            name=f"I-lgl-{_lgl_ctr[0]}",
                            engine=inst.engine,
                            sync_info=mybir.SyncInfo(on_wait=[w], on_update=[]),
                            bass_nofuse=True,
                        ))
                    si.on_wait = waits[-1:]
                    changed = True
                new.append(inst)
            if changed:
                blk.instructions[:] = new


# ------------------------------------------------------------------ device ---
_nc_cache = None


def _build_nc(ns=NS, v_on_gpsimd=V_ON_GPSIMD):
    fd = BC // ns
    nch = fd // 128
    nc = bass.Bass()

    feedT8 = nc.dram_tensor("feedT8", [T // TCHUNK, D, TCHUNK * BC], BF16,
                            kind="ExternalInput")
    wihT = nc.dram_tensor("wihT", [D, G], BF16, kind="ExternalInput")
    whh1 = nc.dram_tensor("whh1", [H, G], BF16, kind="ExternalInput")
    bias2 = nc.dram_tensor("bias2", [1, nch * G], BF16, kind="ExternalInput")
    wout1 = nc.dram_tensor("wout1", [H, 2], BF16, kind="ExternalInput")
    bout1 = nc.dram_tensor("bout1", [2, 1], F32, kind="ExternalInput")
    h0T = nc.dram_tensor("h0T", [H, BC], BF16, kind="ExternalInput")
    c04 = nc.dram_tensor("c04", [128, ns * nch * H], BF16, kind="ExternalInput")
    ident_d = nc.dram_tensor("ident_d", [128, 128], BF16, kind="ExternalInput")
    ones_d = nc.dram_tensor("ones_d", [1, 128], BF16, kind="ExternalInput")
    y_out = nc.dram_tensor("y_out", [2, BC], F32, kind="ExternalOutput")

    with TileContext(nc) as tc:
        with (
            tc.tile_pool(name="const", bufs=1) as cpool,
            tc.tile_pool(name="state", bufs=1) as spool,
            tc.tile_pool(name="feed", bufs=2) as fpool,
            tc.tile_pool(name="acts", bufs=3) as apool,
            tc.tile_pool(name="work", bufs=2) as wpool,
            tc.tile_pool(name="g0", bufs=2, space="PSUM") as gpool0,
            tc.tile_pool(name="g1", bufs=2, space="PSUM") as gpool1,
            tc.tile_pool(name="hps", bufs=1, space="PSUM") as hpool,
        ):
            gpools = [gpool0, gpool1]
            wihT_sb = cpool.tile([D, G], BF16, tag="wihT")
            whh1_sb = cpool.tile([H, G], BF16, tag="whh1")
            bias2_sb = cpool.tile([1, nch * G], BF16, tag="bias2")
            wout1_sb = cpool.tile([H, 2], BF16, tag="wout1")
            bout1_sb = cpool.tile([2, 1], F32, tag="bout1")
            ident = cpool.tile([128, 128], BF16, tag="ident")
            ones_sb = cpool.tile([1, 128], BF16, tag="ones")
            hT4 = [spool.tile([H, nch * 128], BF16, tag=f"hT4_{s}",
                              name=f"hT4_{s}") for s in range(ns)]
            c_sb = [spool.tile([128, nch * H], BF16, tag=f"c{s}",
                               name=f"c{s}") for s in range(ns)]

            nc.sync.dma_start(wihT_sb[:], wihT[:])
            nc.sync.dma_start(whh1_sb[:], whh1[:])
            nc.sync.dma_start(bias2_sb[:], bias2[:])
            nc.sync.dma_start(wout1_sb[:], wout1[:])
            nc.sync.dma_start(bout1_sb[:], bout1[:])
            nc.sync.dma_start(ident[:], ident_d[:])
            nc.sync.dma_start(ones_sb[:], ones_d[:])
            for s in range(ns):
                nc.sync.dma_start(hT4[s][:], h0T[:, s * fd:(s + 1) * fd])
                nc.sync.dma_start(c_sb[s][:],
                                  c04[:, s * nch * H:(s + 1) * nch * H])

            # software-pipelined emission: stream 1 runs half a step behind
            # stream 0 so each engine alternates between the two streams'
            # phases instead of marching them in lockstep.
            fbuf = [None]
            acts = {}

            def emit_p1(s, t):
                tb, tsx = divmod(t, TCHUNK)
                if tsx == 0 and s == 0:
                    fbuf[0] = fpool.tile([D, TCHUNK * BC], BF16, tag="fbuf",
                                         name="fbuf")
                    nc.sync.dma_start(fbuf[0][:], feedT8[tb])
                base = tsx * BC + s * fd
                g_ps = gpools[s].tile([128, nch * G], F32, tag=f"g{s}",
                                      name=f"g{s}")
                for ch in range(nch):
                    sl = slice(ch * G, (ch + 1) * G)
                    feed_t = fbuf[0][:, base + ch * 128: base + (ch + 1) * 128]
                    nc.tensor.matmul(g_ps[:, sl], ones_sb[:],
                                     bias2_sb[:, sl],
                                     start=True, stop=False)
                    nc.tensor.matmul(g_ps[:, sl], feed_t, wihT_sb[:],
                                     start=False, stop=False)
                    nc.tensor.matmul(g_ps[:, sl],
                                     hT4[s][:, ch * 128:(ch + 1) * 128],
                                     whh1_sb[:],
                                     start=False, stop=True)
                a_t = apool.tile([128, nch * G], BF16, tag=f"acts{s}",
                                 name=f"acts{s}")
                av = a_t[:].rearrange("p (c g) -> p c g", c=nch)
                gv = g_ps[:].rearrange("p (c g) -> p c g", c=nch)
                nc.scalar.activation(a_t[:], g_ps[:], SIG)
                acts[s] = av

            def emit_p2(s, t):
                av = acts[s]
                cv = c_sb[s][:].rearrange("p (c h) -> p c h", c=nch)
                u_t = wpool.tile([128, nch * H], BF16, tag=f"u{s}",
                                 name=f"u{s}")
                uv = u_t[:].rearrange("p (c h) -> p c h", c=nch)
                nc.vector.scalar_tensor_tensor(uv, av[:, :, 64:96],
                                               0.5, av[:, :, 0:32],
                                               SUB, MULT)
                v_t = wpool.tile([128, nch * H], BF16, tag=f"v{s}",
                                 name=f"v{s}")
                vv = v_t[:].rearrange("p (c h) -> p c h", c=nch)
                nc.vector.tensor_tensor(vv, av[:, :, 32:64], cv, MULT)
                # state is c/2: (c/2) = f*(c/2) + (g'-0.5)*i  [= v + u]
                nc.vector.tensor_tensor(c_sb[s][:], u_t[:], v_t[:], ADD)

                sig = wpool.tile([128, nch * H], BF16, tag=f"sig{s}",
                                 name=f"sig{s}")
                nc.scalar.activation(sig[:], c_sb[s][:], SIG, scale=4.0)

                m_t = wpool.tile([128, nch * H], BF16, tag=f"m{s}",
                                 name=f"m{s}")
                mv = m_t[:].rearrange("p (c h) -> p c h", c=nch)
                sv = sig[:].rearrange("p (c h) -> p c h", c=nch)
                nc.vector.scalar_tensor_tensor(mv, sv, 0.5,
                                               av[:, :, 96:128],
                                               SUB, MULT)

                hT_ps = hpool.tile([H, nch * 128], BF16, tag=f"hTp{s}",
                                   name=f"hTp{s}")
                for ch in range(nch):
                    nc.tensor.transpose(
                        hT_ps[:, ch * 128:(ch + 1) * 128],
                        m_t[:, ch * H:(ch + 1) * H], ident[:])
                nc.vector.tensor_copy(hT4[s][:], hT_ps[:])

            emit_p1(0, 0)
            for t in range(T):
                emit_p1(1, t)
                emit_p2(0, t)
                if t + 1 < T:
                    emit_p1(0, t + 1)
                emit_p2(1, t)

            y_ps = hpool.tile([2, BC], F32, tag="y")
            for s in range(ns):
                for ch in range(nch):
                    sl = slice(s * fd + ch * 128, s * fd + (ch + 1) * 128)
                    nc.tensor.matmul(y_ps[:, sl],
                                     wout1_sb[:],
                                     hT4[s][:, ch * 128:(ch + 1) * 128],
                                     start=True, stop=True)
            y_sb = wpool.tile([2, BC], F32, tag="ysb")
            nc.scalar.activation(y_sb[:], y_ps[:],
                                 mybir.ActivationFunctionType.Identity,
                                 bias=bout1_sb[:])
            nc.sync.dma_start(y_out[:], y_sb[:])

    _legalize_sync_waits(nc)
    return nc


# -------------------------------------------------------------------- host ---
def _prep_core_inputs(feed_c, W_ih, W_hh, b_ih, b_hh, W_out, b_out, h0_c, c0_c):
    g_rows = slice(64, 96)  # PyTorch gate order i,f,g,o
    nch = BC // NS // 128

    wih_p = W_ih.astype(np.float32).copy()
    wih_p[g_rows] *= 2.0
    wihT = np.ascontiguousarray(wih_p.T).astype(BF)

    whh_p = (2.0 * W_hh.astype(np.float32)).copy()
    whh_p[g_rows] *= 2.0
    whh1 = np.ascontiguousarray(whh_p.T).astype(BF)       # [H, G]

    bias = (b_ih + b_hh).astype(np.float32).copy()
    bias[g_rows] *= 2.0
    bias2 = np.tile(bias, nch)[None, :].astype(BF)        # [1, nch*G]

    wout1 = np.ascontiguousarray(2.0 * W_out.astype(np.float32).T).astype(BF)
    bout1 = np.ascontiguousarray(b_out.astype(np.float32).reshape(2, 1))

    # feed_c [BC, T, D] -> [T/8, D, 8*BC]
    ft = feed_c.transpose(1, 2, 0).reshape(T // TCHUNK, TCHUNK, D, BC)
    feedT8 = np.ascontiguousarray(ft.transpose(0, 2, 1, 3)).reshape(
        T // TCHUNK, D, TCHUNK * BC).astype(BF)

    # h0T [H, BC]: h on partitions, batch on free; h stored as h/2
    h0T = np.ascontiguousarray(h0_c.T / 2.0).astype(BF)
    # c04 [128, ns*nch*H]: batch-major per chunk; state carried as c/2
    c04 = np.ascontiguousarray(
        c0_c.reshape(NS * nch, 128, H).transpose(1, 0, 2).reshape(
            128, NS * nch * H) / 2.0).astype(BF)

    ident = np.eye(128, dtype=np.float32).astype(BF)
    ones = np.ones((1, 128), dtype=np.float32).astype(BF)

    return dict(feedT8=feedT8, wihT=wihT, whh1=whh1, bias2=bias2,
                wout1=wout1, bout1=bout1, h0T=h0T, c04=c04,
                ident_d=ident, ones_d=ones)


def kernel(feed, W_ih, W_hh, b_ih, b_hh, W_out, b_out, h0, c0):
    global _nc_cache, LAST_RESULTS
    feed = np.asarray(feed, dtype=np.float32)
    W_ih = np.asarray(W_ih, dtype=np.float32)
    W_hh = np.asarray(W_hh, dtype=np.float32)
    b_ih = np.asarray(b_ih, dtype=np.float32)
    b_hh = np.asarray(b_hh, dtype=np.float32)
    W_out = np.asarray(W_out, dtype=np.float32)
    b_out = np.asarray(b_out, dtype=np.float32)
    h0 = np.asarray(h0, dtype=np.float32)
    c0 = np.asarray(c0, dtype=np.float32)

    if _nc_cache is None:
        _nc_cache = _build_nc()
    nc = _nc_cache

    in_maps = []
    for c in range(NCORES):
        rows = slice(c * BC, (c + 1) * BC)
        in_maps.append(_prep_core_inputs(
            feed[rows], W_ih, W_hh, b_ih, b_hh, W_out, b_out,
            h0[rows], c0[rows]))

    res = run_bass_kernel_spmd(nc, in_maps, core_ids=list(range(NCORES)))
    LAST_RESULTS = res

    out = np.empty((B, 2), dtype=np.float32)
    for c in range(NCORES):
        out[c * BC:(c + 1) * BC] = res.results[c]["y_out"].T
    return out


# revision 4
# speedup vs baseline: 1.3003x; 1.3003x over previous
"""LSTM discriminator kernel for Trainium2 (8 NeuronCores, SPMD data-parallel).

Problem: B=4096, T=256, D=128, H=32 LSTM + final linear to 2 classes.
Sharding: batch split across 8 cores (512 rows each); weights replicated.

Per-core design v5 (batch-major, 2 batch streams of 256):
  - Per stream, gates PSUM [128b, 2ch x 128g] built by 6 matmuls: per
    chunk a K=1 ones-row matmul (bias), a feed matmul (feed chunk
    stationary), and a K=32 recurrence matmul off hT [32h, 2ch*128b]
    (all operands at SBUF base partition 0 -- base-32 lhsT is a HW bug).
  - One sigmoid ACT covers all gates (tanh folded via doubled g-rows);
    h kept as h/2 (W_hh, W_out doubled); cell state carried as c/2 so
    the update is u=(g'-0.5)*i [STT], v=f*(c/2) [TT 2x],
    c/2=u+v [TT-add 2x]; then sig(4*(c/2)) [ACT] and
    h/2=(sig-0.5)*o [STT].
  - Two PE transposes ([128b, 32h] -> [32h, 128b]) per stream + one
    [32, 256] PSUM->SBUF copy rebuild hT for the next step.
  - Software-pipelined emission of the two streams; per-step period is
    bounded by the 7-hop cross-engine recurrence chain (~3 us).
"""

import numpy as np
import ml_dtypes

import concourse.bass as bass
import concourse.mybir as mybir
from concourse.tile import TileContext
from concourse.bass_utils import run_bass_kernel_spmd

F32 = mybir.dt.float32
BF16 = mybir.dt.bfloat16
FP16 = mybir.dt.float16
BF = ml_dtypes.bfloat16

B, T, D, H = 4096, 256, 128, 32
G = 4 * H          # 128 gate rows
NCORES = 8
BC = B // NCORES   # 512 batch per core
NS = 2             # pipelined batch streams
NCH = BC // NS // 128   # chunks of 128 per stream (=2)
FD = BC // NS      # 256 batch per stream
TCHUNK = 8         # timesteps per feed DMA (1 MiB transfers)
V_ON_GPSIMD = False

SIG = mybir.ActivationFunctionType.Sigmoid
MULT = mybir.AluOpType.mult
SUB = mybir.AluOpType.subtract
ADD = mybir.AluOpType.add

LAST_RESULTS = None


# ---------------------------------------------------------------- legalize ---
_lgl_ctr = [0]


def _legalize_sync_waits(nc):
    for fn in nc.m.functions:
        for blk in fn.blocks:
            new = []
            changed = False
            for inst in blk.instructions:
                si = getattr(inst, "sync_info", None)
                waits = list(si.on_wait) if (si is not None and si.on_wait) else []
                if len(waits) > 1:
                    for w in waits[:-1]:
                        _lgl_ctr[0] += 1
                        new.append(mybir.InstNoOp(
                            name=f"I-lgl-{_lgl_ctr[0]}",
                            engine=inst.engine,
                            sync_info=mybir.SyncInfo(on_wait=[w], on_update=[]),
                            bass_nofuse=True,
                        ))
                    si.on_wait = waits[-1:]
                    changed = True
                new.append(inst)
            if changed:
                blk.instructions[:] = new


# ------------------------------------------------------------------ device ---
_nc_cache = None


def _build_nc(ns=NS, v_on_gpsimd=V_ON_GPSIMD):
    fd = BC // ns
    nch = fd // 128
    nc = bass.Bass()

    feedT8 = nc.dram_tensor("feedT8", [T // TCHUNK, D, TCHUNK * BC], BF16,
                            kind="ExternalInput")
    wihT = nc.dram_tensor("wihT", [D, G], BF16, kind="ExternalInput")
    whh1 = nc.dram_tensor("whh1", [H, G], BF16, kind="ExternalInput")
    bias2 = nc.dram_tensor("bias2", [1, nch * G], BF16, kind="ExternalInput")
    wout1 = nc.dram_tensor("wout1", [H, 2], BF16, kind="ExternalInput")
    bout1 = nc.dram_tensor("bout1", [2, 1], F32, kind="ExternalInput")
    h0T = nc.dram_tensor("h0T", [H, BC], BF16, kind="ExternalInput")
    c04 = nc.dram_tensor("c04", [128, ns * nch * H], BF16, kind="ExternalInput")
    ident_d = nc.dram_tensor("ident_d", [128, 128], BF16, kind="ExternalInput")
    ones_d = nc.dram_tensor("ones_d", [1, 128], BF16, kind="ExternalInput")
    y_out = nc.dram_tensor("y_out", [2, BC], F32, kind="ExternalOutput")

    with TileContext(nc) as tc:
        with (
            tc.tile_pool(name="const", bufs=1) as cpool,
            tc.tile_pool(name="state", bufs=1) as spool,
            tc.tile_pool(name="feed", bufs=2) as fpool,
            tc.tile_pool(name="acts", bufs=3) as apool,
            tc.tile_pool(name="work", bufs=2) as wpool,
            tc.tile_pool(name="g0", bufs=2, space="PSUM") as gpool0,
            tc.tile_pool(name="g1", bufs=2, space="PSUM") as gpool1,
            tc.tile_pool(name="hps", bufs=1, space="PSUM") as hpool,
        ):
            gpools = [gpool0, gpool1]
            wihT_sb = cpool.tile([D, G], BF16, tag="wihT")
            whh1_sb = cpool.tile([H, G], BF16, tag="whh1")
            bias2_sb = cpool.tile([1, nch * G], BF16, tag="bias2")
            wout1_sb = cpool.tile([H, 2], BF16, tag="wout1")
            bout1_sb = cpool.tile([2, 1], F32, tag="bout1")
            ident = cpool.tile([128, 128], BF16, tag="ident")
            ones_sb = cpool.tile([1, 128], BF16, tag="ones")
            hT4 = [spool.tile([H, nch * 128], BF16, tag=f"hT4_{s}",
                              name=f"hT4_{s}") for s in range(ns)]
            c_sb = [spool.tile([128, nch * H], BF16, tag=f"c{s}",
                               name=f"c{s}") for s in range(ns)]

            nc.sync.dma_start(wihT_sb[:], wihT[:])
            nc.sync.dma_start(whh1_sb[:], whh1[:])
            nc.sync.dma_start(bias2_sb[:], bias2[:])
            nc.sync.dma_start(wout1_sb[:], wout1[:])
            nc.sync.dma_start(bout1_sb[:], bout1[:])
            nc.sync.dma_start(ident[:], ident_d[:])
            nc.sync.dma_start(ones_sb[:], ones_d[:])
            for s in range(ns):
                nc.sync.dma_start(hT4[s][:], h0T[:, s * fd:(s + 1) * fd])
                nc.sync.dma_start(c_sb[s][:],
                                  c04[:, s * nch * H:(s + 1) * nch * H])

            # software-pipelined emission: stream 1 runs half a step behind
            # stream 0 so each engine alternates between the two streams'
            # phases instead of marching them in lockstep.
            fbuf = [None]
            acts = {}

            def emit_p1(s, t):
                tb, tsx = divmod(t, TCHUNK)
                if tsx == 0 and s == 0:
                    fbuf[0] = fpool.tile([D, TCHUNK * BC], BF16, tag="fbuf",
                                         name="fbuf")
                    nc.sync.dma_start(fbuf[0][:], feedT8[tb])
                base = tsx * BC + s * fd
                g_ps = gpools[s].tile([128, nch * G], F32, tag=f"g{s}",
                                      name=f"g{s}")
                for ch in range(nch):
                    sl = slice(ch * G, (ch + 1) * G)
                    feed_t = fbuf[0][:, base + ch * 128: base + (ch + 1) * 128]
                    nc.tensor.matmul(g_ps[:, sl], ones_sb[:],
                                     bias2_sb[:, sl],
                                     start=True, stop=False)
                    nc.tensor.matmul(g_ps[:, sl], feed_t, wihT_sb[:],
                                     start=False, stop=False)
                    nc.tensor.matmul(g_ps[:, sl],
                                     hT4[s][:, ch * 128:(ch + 1) * 128],
                                     whh1_sb[:],
                                     start=False, stop=True)
                a_t = apool.tile([128, nch * G], BF16, tag=f"acts{s}",
                                 name=f"acts{s}")
                av = a_t[:].rearrange("p (c g) -> p c g", c=nch)
                gv = g_ps[:].rearrange("p (c g) -> p c g", c=nch)
                nc.scalar.activation(a_t[:], g_ps[:], SIG)
                acts[s] = av

            def emit_p2(s, t):
                av = acts[s]
                cv = c_sb[s][:].rearrange("p (c h) -> p c h", c=nch)
                u_t = wpool.tile([128, nch * H], BF16, tag=f"u{s}",
                                 name=f"u{s}")
                uv = u_t[:].rearrange("p (c h) -> p c h", c=nch)
                nc.vector.scalar_tensor_tensor(uv, av[:, :, 64:96],
                                               0.5, av[:, :, 0:32],
                                               SUB, MULT)
                v_t = wpool.tile([128, nch * H], BF16, tag=f"v{s}",
                                 name=f"v{s}")
                vv = v_t[:].rearrange("p (c h) -> p c h", c=nch)
                nc.vector.tensor_tensor(vv, av[:, :, 32:64], cv, MULT)
                # state is c/2: (c/2) = f*(c/2) + (g'-0.5)*i  [= v + u]
                nc.vector.tensor_tensor(c_sb[s][:], u_t[:], v_t[:], ADD)

                sig = wpool.tile([128, nch * H], BF16, tag=f"sig{s}",
                                 name=f"sig{s}")
                nc.scalar.activation(sig[:], c_sb[s][:], SIG, scale=4.0)

                m_t = wpool.tile([128, nch * H], BF16, tag=f"m{s}",
                                 name=f"m{s}")
                mv = m_t[:].rearrange("p (c h) -> p c h", c=nch)
                sv = sig[:].rearrange("p (c h) -> p c h", c=nch)
                nc.vector.scalar_tensor_tensor(mv, sv, 0.5,
                                               av[:, :, 96:128],
                                               SUB, MULT)

                hT_ps = hpool.tile([H, nch * 128], BF16, tag=f"hTp{s}",
                                   name=f"hTp{s}")
                for ch in range(nch):
                    nc.tensor.transpose(
                        hT_ps[:, ch * 128:(ch + 1) * 128],
                        m_t[:, ch * H:(ch + 1) * H], ident[:])
                nc.vector.tensor_copy(hT4[s][:], hT_ps[:])

            emit_p1(0, 0)
            for t in range(T):
                emit_p1(1, t)
                emit_p2(0, t)
                if t + 1 < T:
                    emit_p1(0, t + 1)
                emit_p2(1, t)

            y_ps = hpool.tile([2, BC], F32, tag="y")
            for s in range(ns):
                for ch in range(nch):
                    sl = slice(s * fd + ch * 128, s * fd + (ch + 1) * 128)
                    nc.tensor.matmul(y_ps[:, sl],
                                     wout1_sb[:],
                                     hT4[s][:, ch * 128:(ch + 1) * 128],
                                     start=True, stop=True)
            y_sb = wpool.tile([2, BC], F32, tag="ysb")
            nc.scalar.activation(y_sb[:], y_ps[:],
                                 mybir.ActivationFunctionType.Identity,
                                 bias=bout1_sb[:])
            nc.sync.dma_start(y_out[:], y_sb[:])

    _legalize_sync_waits(nc)
    return nc


# -------------------------------------------------------------------- host ---
def _prep_core_inputs(feed_c, W_ih, W_hh, b_ih, b_hh, W_out, b_out, h0_c, c0_c):
    g_rows = slice(64, 96)  # PyTorch gate order i,f,g,o
    nch = BC // NS // 128

    wih_p = W_ih.astype(np.float32).copy()
    wih_p[g_rows] *= 2.0
    wihT = np.ascontiguousarray(wih_p.T).astype(BF)

    whh_p = (2.0 * W_hh.astype(np.float32)).copy()
    whh_p[g_rows] *= 2.0
    whh1 = np.ascontiguousarray(whh_p.T).astype(BF)       # [H, G]

    bias = (b_ih + b_hh).astype(np.float32).copy()
    bias[g_rows] *= 2.0
    bias2 = np.tile(bias, nch)[None, :].astype(BF)        # [1, nch*G]

    wout1 = np.ascontiguousarray(2.0 * W_out.astype(np.float32).T).astype(BF)
    bout1 = np.ascontiguousarray(b_out.astype(np.float32).reshape(2, 1))

    # feed_c [BC, T, D] -> [T/8, D, 8*BC]
    ft = feed_c.transpose(1, 2, 0).reshape(T // TCHUNK, TCHUNK, D, BC)
    feedT8 = np.ascontiguousarray(ft.transpose(0, 2, 1, 3)).reshape(
        T // TCHUNK, D, TCHUNK * BC).astype(BF)

    # h0T [H, BC]: h on partitions, batch on free; h stored as h/2
    h0T = np.ascontiguousarray(h0_c.T / 2.0).astype(BF)
    # c04 [128, ns*nch*H]: batch-major per chunk; state carried as c/2
    c04 = np.ascontiguousarray(
        c0_c.reshape(NS * nch, 128, H).transpose(1, 0, 2).reshape(
            128, NS * nch * H) / 2.0).astype(BF)

    ident = np.eye(128, dtype=np.float32).astype(BF)
    ones = np.ones((1, 128), dtype=np.float32).astype(BF)

    return dict(feedT8=feedT8, wihT=wihT, whh1=whh1, bias2=bias2,
                wout1=wout1, bout1=bout1, h0T=h0T, c04=c04,
                ident_d=ident, ones_d=ones)


def kernel(feed, W_ih, W_hh, b_ih, b_hh, W_out, b_out, h0, c0):
    global _nc_cache, LAST_RESULTS
    feed = np.asarray(feed, dtype=np.float32)
    W_ih = np.asarray(W_ih, dtype=np.float32)
    W_hh = np.asarray(W_hh, dtype=np.float32)
    b_ih = np.asarray(b_ih, dtype=np.float32)
    b_hh = np.asarray(b_hh, dtype=np.float32)
    W_out = np.asarray(W_out, dtype=np.float32)
    b_out = np.asarray(b_out, dtype=np.float32)
    h0 = np.asarray(h0, dtype=np.float32)
    c0 = np.asarray(c0, dtype=np.float32)

    if _nc_cache is None:
        _nc_cache = _build_nc()
    nc = _nc_cache

    in_maps = []
    for c in range(NCORES):
        rows = slice(c * BC, (c + 1) * BC)
        in_maps.append(_prep_core_inputs(
            feed[rows], W_ih, W_hh, b_ih, b_hh, W_out, b_out,
            h0[rows], c0[rows]))

    res = run_bass_kernel_spmd(nc, in_maps, core_ids=list(range(NCORES)))
    LAST_RESULTS = res

    out = np.empty((B, 2), dtype=np.float32)
    for c in range(NCORES):
        out[c * BC:(c + 1) * BC] = res.results[c]["y_out"].T
    return out
